# revision 41
# baseline (speedup 1.0000x reference)
"""Trainium2 Bass kernel for nn_AttentionalReadout (segment-softmax pooling).

Algorithm (8-core SPMD, data-parallel over nodes):
  gate_i = tanh(x_i @ W1 + b1) @ W2            (per node, MLP on device)
  e_i    = exp(gate_i)                          (b2 and the segment max cancel
                                                 in softmax; gate is bounded by
                                                 sum|W2| ~ 11.4 so exp is safe)
  out[g] = sum_i e_i x_i / sum_i e_i            (per graph)

Device strategy per core (v2 — no PE transposes):
  - nodes sharded at graph boundaries across the 8 cores (host plan)
  - x is uploaded TWICE in different layouts: node-major bf16 (for the
    pooling matmul, which contracts over nodes) and feature-major fp8_e4m3
    (for the gate MLP, which contracts over features). This removes the
    8 PE transposes + PSUM->SBUF copies per 4-tile group of the v1 kernel
    at the cost of +256 B/node of DMA.
  - the block-local graph index lidx is streamed as bf16 (2 B/node) and the
    one-hot matrix is built on DVE via is_equal against an iota constant,
    then scaled by e in place; padded rows have lidx = -1 (zero row).
  - per block of TB tiles a single accumulating PE matmul computes
    U[g, :] = E^T @ [x | 1] in PSUM, giving weighted feature sums and the
    softmax denominators; host sums partials of graphs straddling
    block/core boundaries and divides.
"""

import numpy as np

import concourse.bacc as bacc
import concourse.tile as tile
import concourse.mybir as mybir
from concourse.bass_utils import run_bass_kernel_spmd

P = 128            # nodes per tile (partition dim)
HDIM = 256         # node feature dim
HHID = 128         # gate MLP hidden dim
NUM_GRAPHS = 8192
N_CORES = 8
GROUP = 4          # tiles batched per tanh/exp activation

_FP = mybir.dt.float32
_BF = mybir.dt.bfloat16
_F8 = mybir.dt.float8e4
_NP_BF = mybir.dt.np(_BF)
_NP_F8 = mybir.dt.np(_F8)


def _plan(batch):
    """Choose node ranges per core and the uniform block geometry."""
    gpc = NUM_GRAPHS // N_CORES
    bounds = np.searchsorted(
        batch, np.arange(N_CORES + 1, dtype=np.int64) * gpc, side="left"
    ).astype(np.int64)
    t_need = max(1, int(np.ceil(np.diff(bounds).max() / P)))
    for tb, g_blk in [(32, 64), (16, 64), (16, 128), (8, 128), (4, 128)]:
        w = tb * P
        ok = True
        for c in range(N_CORES):
            s, e = int(bounds[c]), int(bounds[c + 1])
            nb = int(np.ceil(max(e - s, 0) / w))
            for j in range(nb):
                lo = s + j * w
                hi = min(lo + w, e)
                if hi <= lo:
                    continue
                if int(batch[hi - 1]) - int(batch[lo]) >= g_blk:
                    ok = False
                    break
            if not ok:
                break
        if ok:
            n_blocks = int(np.ceil(t_need / tb))
            return bounds, tb, g_blk, n_blocks, n_blocks * tb
    raise ValueError("no valid block plan for this batch vector")


def _build_program(T, TB, G_BLK, B):
    """Build the SPMD Bass program (identical across cores)."""
    nc = bacc.Bacc("TRN2", target_bir_lowering=False, debug=False)
    # node-major [x | 1] rows, bf16, per-partition-contiguous per block
    xn_d = nc.dram_tensor("xn", [B, P, TB * (HDIM + 1)], _BF, kind="ExternalInput")
    # feature-major x, fp8: xtq[j, p, c*TB*P + n] = x[block j node n, 128c + p]
    xtq_d = nc.dram_tensor("xtq", [B, P, 2 * TB * P], _F8, kind="ExternalInput")
    # block-local graph index per node, -1 for padding (persistent, loaded once)
    lidx_d = nc.dram_tensor("lidx", [P, B * TB], _BF, kind="ExternalInput")
    cf_d = nc.dram_tensor("cf", [P, 1], _FP, kind="ExternalInput")  # b1
    # bf16 consts: [0:G_BLK] iota over graphs, [G_BLK] W2
    cb_d = nc.dram_tensor("cb", [P, G_BLK + 1], _BF, kind="ExternalInput")
    # fp8 consts: W1 chunks: [0:128] W1[:128,:], [128:256] W1[128:,:]
    cq_d = nc.dram_tensor("cq", [P, 2 * HHID], _F8, kind="ExternalInput")
    # column-tile partials per block (2 if they fit in 128 PSUM partitions),
    # summed on host
    NPART = 2 if 2 * G_BLK <= P else 1
    out_d = nc.dram_tensor("out", [B, G_BLK, HDIM + 1], _FP, kind="ExternalOutput")

    Tanh = mybir.ActivationFunctionType.Tanh
    Exp = mybir.ActivationFunctionType.Exp
    EQ = mybir.AluOpType.is_equal
    MUL = mybir.AluOpType.mult

    with tile.TileContext(nc) as tc:
        with (
            tc.tile_pool(name="const", bufs=1) as const_pool,
            tc.tile_pool(name="xn", bufs=2) as xn_pool,
            tc.tile_pool(name="xq", bufs=2) as xq_pool,
            tc.tile_pool(name="E", bufs=2) as E_pool,
            tc.tile_pool(name="es", bufs=2) as es_pool,
            tc.tile_pool(name="u", bufs=3) as u_pool,
            tc.tile_pool(name="osb", bufs=2) as o_pool,
            tc.tile_pool(name="hp", bufs=2, space="PSUM") as h_pool,
            tc.tile_pool(name="gp", bufs=2, space="PSUM") as gate_pool,
            tc.tile_pool(name="Up", bufs=2, space="PSUM") as U_pool,
        ):
            cf = const_pool.tile([P, 1], _FP)
            nc.sync.dma_start(cf[:], cf_d.ap()[:])
            cb = const_pool.tile([P, G_BLK + 1], _BF)
            nc.sync.dma_start(cb[:], cb_d.ap()[:])
            cq = const_pool.tile([P, 2, HHID], _F8)
            nc.sync.dma_start(cq[:], cq_d.ap()[:].rearrange("p (c h) -> p c h", c=2))
            b1c = cf[:, 0:1]
            iota_g = cb[:, 0:G_BLK]
            w2c = cb[:, G_BLK : G_BLK + 1]
            lidx_all = const_pool.tile([P, B * TB], _BF)
            nc.sync.dma_start(lidx_all[:], lidx_d.ap()[:])

            for j in range(B):
                # raw one-hot from lidx (overlaps pass A; es scales it later)
                E_sb = E_pool.tile([P, TB, G_BLK], _BF)
                nc.vector.tensor_tensor(
                    E_sb[:],
                    lidx_all[:, j * TB : (j + 1) * TB, None].to_broadcast(
                        [P, TB, G_BLK]
                    ),
                    iota_g[:, None, :].to_broadcast([P, TB, G_BLK]),
                    EQ,
                )
                xtq_sb = xq_pool.tile([P, 2, TB * P], _F8)
                nc.sync.dma_start(
                    xtq_sb[:], xtq_d.ap()[j].rearrange("p (c n) -> p c n", c=2)
                )
                xn_sb = xn_pool.tile([P, TB, HDIM + 1], _BF)
                # chunk so each DMA moves ~8 KB contiguous per-partition lines
                xn_v = xn_d.ap()[j].rearrange("p (t f) -> p t f", t=TB)
                if TB * (HDIM + 1) * 2 <= 8224:
                    nc.sync.dma_start(xn_sb[:], xn_v)
                else:
                    hb = TB // 2
                    nc.sync.dma_start(xn_sb[:, 0:hb, :], xn_v[:, 0:hb, :])
                    nc.sync.dma_start(xn_sb[:, hb:TB, :], xn_v[:, hb:TB, :])
                es = es_pool.tile([P, TB], _BF)
                # pass A: gate MLP per group of GROUP tiles
                for g in range(TB // GROUP):
                    n0, n1 = g * GROUP * P, (g + 1) * GROUP * P
                    h_ps = h_pool.tile([P, GROUP * HHID], _FP)
                    nc.tensor.matmul(
                        h_ps[:], cq[:, 0, :], xtq_sb[:, 0, n0:n1], start=True, stop=False
                    )
                    nc.tensor.matmul(
                        h_ps[:], cq[:, 1, :], xtq_sb[:, 1, n0:n1], start=False, stop=True
                    )
                    u_sb = u_pool.tile([P, GROUP * HHID], _BF)
                    nc.scalar.activation(u_sb[:], h_ps[:], Tanh, bias=b1c)
                    gate_ps = gate_pool.tile([P, GROUP], _FP)
                    for q in range(GROUP):
                        nc.tensor.matmul(
                            gate_ps[:, q : q + 1],
                            u_sb[:, q * HHID : (q + 1) * HHID],
                            w2c,
                            start=True,
                            stop=True,
                        )
                    nc.scalar.activation(
                        es[:, g * GROUP : (g + 1) * GROUP], gate_ps[:], Exp
                    )
                    # scale this group's one-hot rows by e in place
                    nc.vector.tensor_tensor(
                        E_sb[:, g * GROUP : (g + 1) * GROUP, :],
                        E_sb[:, g * GROUP : (g + 1) * GROUP, :],
                        es[:, g * GROUP : (g + 1) * GROUP, None].to_broadcast(
                            [P, GROUP, G_BLK]
                        ),
                        MUL,
                    )
                # pass B: weighted one-hot accumulation, column-tiled when the
                # two partials fit in one PSUM tile — even tiles accumulate on
                # PE columns 0:G_BLK, odd tiles on G_BLK:2*G_BLK, concurrently
                U_ps = U_pool.tile([NPART * G_BLK, HDIM + 1], _FP)
                for t in range(TB):
                    col = t % NPART
                    nc.tensor.matmul(
                        U_ps[col * G_BLK : (col + 1) * G_BLK, :],
                        E_sb[:, t, :],
                        xn_sb[:, t, :],
                        start=(t == col),
                        stop=(t == TB - NPART + col),
                        tile_position=(0, col * G_BLK) if NPART == 2 else None,
                    )
                out_sb = o_pool.tile([G_BLK, HDIM + 1], _FP)
                nc.scalar.copy(out_sb[:], U_ps[0:G_BLK, :])
                if NPART == 2:
                    # fold the second column-tile partial in on DVE
                    nc.vector.tensor_tensor(
                        out_sb[:],
                        out_sb[:],
                        U_ps[G_BLK : 2 * G_BLK, :],
                        mybir.AluOpType.add,
                    )
                # out-DMA on the Activation HWDGE ring so it cannot head-of-line
                # block the input prefetch descriptors on the sync ring
                nc.scalar.dma_start(out_d.ap()[j], out_sb[:])

    nc.compile()
    return nc


def _prep_core(x, batch, bounds, c, T, TB, G_BLK):
    """Per-core padded shards in the three device layouts + per-block bases."""
    s, e = int(bounds[c]), int(bounds[c + 1])
    n = e - s
    B = T // TB
    w = TB * P
    x_c = np.zeros((T * P, HDIM), dtype=np.float32)
    x_c[:n] = x[s:e]
    # node-major [x | 1] bf16: [B, P, TB*(HDIM+1)]
    xn1 = np.ones((T * P, HDIM + 1), dtype=_NP_BF)
    xn1[:, :HDIM] = x_c.astype(_NP_BF)
    xn = np.ascontiguousarray(
        xn1.reshape(B, TB, P, HDIM + 1).transpose(0, 2, 1, 3)
    ).reshape(B, P, TB * (HDIM + 1))
    # feature-major fp8: [B, P, 2*TB*P]
    xtq = np.ascontiguousarray(
        x_c.astype(_NP_F8).reshape(B, TB * P, 2, HHID).transpose(0, 3, 2, 1)
    ).reshape(B, P, 2 * TB * P)
    lidx = np.full(T * P, -1, dtype=np.int64)
    g0 = np.zeros(B, dtype=np.int64)
    bl = batch[s:e]
    for j in range(B):
        lo = j * w
        hi = min(lo + w, n)
        if hi <= lo:
            g0[j] = int(batch[e - 1]) if n > 0 else 0
            continue
        g0[j] = int(bl[lo])
        lidx[lo:hi] = bl[lo:hi] - g0[j]
    lidx_b = np.ascontiguousarray(lidx.astype(_NP_BF).reshape(T, P).transpose(1, 0))
    return xn, xtq, lidx_b, g0


def _make_consts(W1, b1, W2, G_BLK):
    cf = np.zeros((P, 1), dtype=np.float32)
    cf[:, 0] = b1
    cb = np.zeros((P, G_BLK + 1), dtype=_NP_BF)
    cb[:, 0:G_BLK] = np.arange(G_BLK, dtype=np.float32)[None, :].astype(_NP_BF)
    cb[:, G_BLK] = W2[:, 0].astype(_NP_BF)
    cq = np.zeros((P, 2 * HHID), dtype=_NP_F8)
    cq[:, 0:HHID] = W1[:HHID, :].astype(_NP_F8)
    cq[:, HHID : 2 * HHID] = W1[HHID:, :].astype(_NP_F8)
    return cf, cb, cq


_CACHE = {}


def _get_program(T, TB, G_BLK, B):
    key = (T, TB, G_BLK, B)
    if key not in _CACHE:
        _CACHE[key] = _build_program(T, TB, G_BLK, B)
    return _CACHE[key]


def build_in_maps(x, W1, b1, W2, batch):
    """Host-side prep shared by kernel() and the timing harness."""
    batch = np.asarray(batch, dtype=np.int64)
    x = np.asarray(x, dtype=np.float32)
    bounds, TB, G_BLK, B, T = _plan(batch)
    cf, cb, cq = _make_consts(
        np.asarray(W1, dtype=np.float32),
        np.asarray(b1, dtype=np.float32),
        np.asarray(W2, dtype=np.float32),
        G_BLK,
    )
    in_maps, g0s = [], []
    for c in range(N_CORES):
        xn, xtq, lidx_b, g0 = _prep_core(x, batch, bounds, c, T, TB, G_BLK)
        in_maps.append(
            {"xn": xn, "xtq": xtq, "lidx": lidx_b, "cf": cf, "cb": cb, "cq": cq}
        )
        g0s.append(g0)
    return in_maps, g0s, (T, TB, G_BLK, B)


def combine(results, g0s, G_BLK):
    """Sum per-block partials into the global output and normalize."""
    U = np.zeros((NUM_GRAPHS + G_BLK, HDIM), dtype=np.float64)
    S = np.zeros(NUM_GRAPHS + G_BLK, dtype=np.float64)
    for out_c, g0 in zip(results, g0s):
        npart = out_c.shape[1] // G_BLK
        oc = out_c.reshape(out_c.shape[0], npart, G_BLK, HDIM + 1).sum(axis=1)
        for j in range(oc.shape[0]):
            g = int(g0[j])
            U[g : g + G_BLK] += oc[j, :, :HDIM]
            S[g : g + G_BLK] += oc[j, :, HDIM]
    return (U[:NUM_GRAPHS] / (S[:NUM_GRAPHS, None] + 1e-16)).astype(np.float32)


def kernel(x, W1, b1, W2, b2, batch):
    in_maps, g0s, (T, TB, G_BLK, B) = build_in_maps(x, W1, b1, W2, batch)
    nc = _get_program(T, TB, G_BLK, B)
    res = run_bass_kernel_spmd(nc, in_maps, core_ids=list(range(N_CORES)))
    outs = [res.results[c]["out"] for c in range(N_CORES)]
    return combine(outs, g0s, G_BLK)


# revision 47
# speedup vs baseline: 1.0738x; 1.0738x over previous
"""Trainium2 Bass kernel for nn_AttentionalReadout (segment-softmax pooling).

Algorithm (8-core SPMD, data-parallel over nodes):
  gate_i = tanh(x_i @ W1 + b1) @ W2            (per node, MLP on device)
  e_i    = exp(gate_i)                          (b2 and the segment max cancel
                                                 in softmax; gate is bounded by
                                                 sum|W2| ~ 11.4 so exp is safe)
  out[g] = sum_i e_i x_i / sum_i e_i            (per graph)

Device strategy per core (v2 — no PE transposes):
  - nodes sharded at graph boundaries across the 8 cores (host plan)
  - x is uploaded TWICE in different layouts: node-major bf16 (for the
    pooling matmul, which contracts over nodes) and feature-major fp8_e4m3
    (for the gate MLP, which contracts over features). This removes the
    8 PE transposes + PSUM->SBUF copies per 4-tile group of the v1 kernel
    at the cost of +256 B/node of DMA.
  - the block-local graph index lidx is streamed as bf16 (2 B/node) and the
    one-hot matrix is built on DVE via is_equal against an iota constant,
    then scaled by e in place; padded rows have lidx = -1 (zero row).
  - per block of TB tiles a single accumulating PE matmul computes
    U[g, :] = E^T @ [x | 1] in PSUM, giving weighted feature sums and the
    softmax denominators; host sums partials of graphs straddling
    block/core boundaries and divides.
"""

import numpy as np

import concourse.bacc as bacc
import concourse.tile as tile
import concourse.mybir as mybir
from concourse.bass_utils import run_bass_kernel_spmd

P = 128            # nodes per tile (partition dim)
HDIM = 256         # node feature dim
HHID = 128         # gate MLP hidden dim
NUM_GRAPHS = 8192
N_CORES = 8
GROUP = 4          # tiles batched per tanh/exp activation

_FP = mybir.dt.float32
_BF = mybir.dt.bfloat16
_F8 = mybir.dt.float8e4
_NP_BF = mybir.dt.np(_BF)
_NP_F8 = mybir.dt.np(_F8)


def _plan(batch):
    """Choose node ranges per core and the uniform block geometry."""
    gpc = NUM_GRAPHS // N_CORES
    bounds = np.searchsorted(
        batch, np.arange(N_CORES + 1, dtype=np.int64) * gpc, side="left"
    ).astype(np.int64)
    t_need = max(1, int(np.ceil(np.diff(bounds).max() / P)))
    for tb, g_blk in [(32, 64), (16, 64), (16, 128), (8, 128), (4, 128)]:
        w = tb * P
        ok = True
        for c in range(N_CORES):
            s, e = int(bounds[c]), int(bounds[c + 1])
            nb = int(np.ceil(max(e - s, 0) / w))
            for j in range(nb):
                lo = s + j * w
                hi = min(lo + w, e)
                if hi <= lo:
                    continue
                if int(batch[hi - 1]) - int(batch[lo]) >= g_blk:
                    ok = False
                    break
            if not ok:
                break
        if ok:
            n_blocks = int(np.ceil(t_need / tb))
            return bounds, tb, g_blk, n_blocks, n_blocks * tb
    raise ValueError("no valid block plan for this batch vector")


def _build_program(T, TB, G_BLK, B):
    """Build the SPMD Bass program (identical across cores)."""
    nc = bacc.Bacc("TRN2", target_bir_lowering=False, debug=False)
    # node-major [x | 1] rows, bf16, per-partition-contiguous per block
    xn_d = nc.dram_tensor("xn", [B, P, TB * (HDIM + 1)], _BF, kind="ExternalInput")
    # feature-major x, fp8: xtq[j, p, c*TB*P + n] = x[block j node n, 128c + p]
    xtq_d = nc.dram_tensor("xtq", [B, P, 2 * TB * P], _F8, kind="ExternalInput")
    # block-local graph index per node, -1 for padding
    lidx_d = nc.dram_tensor("lidx", [B, P, TB], _BF, kind="ExternalInput")
    cf_d = nc.dram_tensor("cf", [P, 1], _FP, kind="ExternalInput")  # b1
    # bf16 consts: [0:G_BLK] iota over graphs, [G_BLK] W2
    cb_d = nc.dram_tensor("cb", [P, G_BLK + 1], _BF, kind="ExternalInput")
    # fp8 consts: W1 chunks: [0:128] W1[:128,:], [128:256] W1[128:,:]
    cq_d = nc.dram_tensor("cq", [P, 2 * HHID], _F8, kind="ExternalInput")
    # column-tile partials per block (2 if they fit in 128 PSUM partitions),
    # summed on host
    NPART = 2 if 2 * G_BLK <= P else 1
    out_d = nc.dram_tensor(
        "out", [B, NPART * G_BLK, HDIM + 1], _FP, kind="ExternalOutput"
    )

    Tanh = mybir.ActivationFunctionType.Tanh
    Exp = mybir.ActivationFunctionType.Exp
    EQ = mybir.AluOpType.is_equal
    MUL = mybir.AluOpType.mult

    with tile.TileContext(nc) as tc:
        with (
            tc.tile_pool(name="const", bufs=1) as const_pool,
            tc.tile_pool(name="xn", bufs=2) as xn_pool,
            tc.tile_pool(name="xq", bufs=2) as xq_pool,
            tc.tile_pool(name="lidx", bufs=2) as lidx_pool,
            tc.tile_pool(name="E", bufs=2) as E_pool,
            tc.tile_pool(name="es", bufs=2) as es_pool,
            tc.tile_pool(name="u", bufs=3) as u_pool,
            tc.tile_pool(name="osb", bufs=2) as o_pool,
            tc.tile_pool(name="hp", bufs=2, space="PSUM") as h_pool,
            tc.tile_pool(name="gp", bufs=2, space="PSUM") as gate_pool,
            tc.tile_pool(name="Up", bufs=2, space="PSUM") as U_pool,
        ):
            cf = const_pool.tile([P, 1], _FP)
            nc.sync.dma_start(cf[:], cf_d.ap()[:])
            cb = const_pool.tile([P, G_BLK + 1], _BF)
            nc.sync.dma_start(cb[:], cb_d.ap()[:])
            cq = const_pool.tile([P, 2, HHID], _F8)
            nc.sync.dma_start(cq[:], cq_d.ap()[:].rearrange("p (c h) -> p c h", c=2))
            b1c = cf[:, 0:1]
            iota_g = cb[:, 0:G_BLK]
            w2c = cb[:, G_BLK : G_BLK + 1]

            for j in range(B):
                lidx_sb = lidx_pool.tile([P, TB], _BF)
                nc.sync.dma_start(lidx_sb[:], lidx_d.ap()[j])
                # raw one-hot from lidx (overlaps pass A; es scales it later)
                E_sb = E_pool.tile([P, TB, G_BLK], _BF)
                nc.vector.tensor_tensor(
                    E_sb[:],
                    lidx_sb[:, :, None].to_broadcast([P, TB, G_BLK]),
                    iota_g[:, None, :].to_broadcast([P, TB, G_BLK]),
                    EQ,
                )
                xtq_sb = xq_pool.tile([P, 2, TB * P], _F8)
                nc.sync.dma_start(
                    xtq_sb[:], xtq_d.ap()[j].rearrange("p (c n) -> p c n", c=2)
                )
                xn_sb = xn_pool.tile([P, TB, HDIM + 1], _BF)
                # chunk so each DMA moves ~8 KB contiguous per-partition lines
                xn_v = xn_d.ap()[j].rearrange("p (t f) -> p t f", t=TB)
                if TB * (HDIM + 1) * 2 <= 8224:
                    nc.sync.dma_start(xn_sb[:], xn_v)
                else:
                    hb = TB // 2
                    nc.sync.dma_start(xn_sb[:, 0:hb, :], xn_v[:, 0:hb, :])
                    nc.sync.dma_start(xn_sb[:, hb:TB, :], xn_v[:, hb:TB, :])
                es = es_pool.tile([P, TB], _BF)
                # pass A: gate MLP per group of GROUP tiles
                for g in range(TB // GROUP):
                    n0, n1 = g * GROUP * P, (g + 1) * GROUP * P
                    h_ps = h_pool.tile([P, GROUP * HHID], _FP)
                    nc.tensor.matmul(
                        h_ps[:], cq[:, 0, :], xtq_sb[:, 0, n0:n1], start=True, stop=False
                    )
                    nc.tensor.matmul(
                        h_ps[:], cq[:, 1, :], xtq_sb[:, 1, n0:n1], start=False, stop=True
                    )
                    u_sb = u_pool.tile([P, GROUP * HHID], _BF)
                    nc.scalar.activation(u_sb[:], h_ps[:], Tanh, bias=b1c)
                    gate_ps = gate_pool.tile([P, GROUP], _FP)
                    for q in range(GROUP):
                        nc.tensor.matmul(
                            gate_ps[:, q : q + 1],
                            u_sb[:, q * HHID : (q + 1) * HHID],
                            w2c,
                            start=True,
                            stop=True,
                        )
                    nc.scalar.activation(
                        es[:, g * GROUP : (g + 1) * GROUP], gate_ps[:], Exp
                    )
                    # scale this group's one-hot rows by e in place
                    nc.vector.tensor_tensor(
                        E_sb[:, g * GROUP : (g + 1) * GROUP, :],
                        E_sb[:, g * GROUP : (g + 1) * GROUP, :],
                        es[:, g * GROUP : (g + 1) * GROUP, None].to_broadcast(
                            [P, GROUP, G_BLK]
                        ),
                        MUL,
                    )
                # pass B: weighted one-hot accumulation, column-tiled when the
                # two partials fit in one PSUM tile — even tiles accumulate on
                # PE columns 0:G_BLK, odd tiles on G_BLK:2*G_BLK, concurrently
                U_ps = U_pool.tile([NPART * G_BLK, HDIM + 1], _FP)
                for t in range(TB):
                    col = t % NPART
                    nc.tensor.matmul(
                        U_ps[col * G_BLK : (col + 1) * G_BLK, :],
                        E_sb[:, t, :],
                        xn_sb[:, t, :],
                        start=(t == col),
                        stop=(t == TB - NPART + col),
                        tile_position=(0, col * G_BLK) if NPART == 2 else None,
                    )
                out_sb = o_pool.tile([NPART * G_BLK, HDIM + 1], _FP)
                nc.scalar.copy(out_sb[:], U_ps[:])
                # out-DMA on the Activation HWDGE ring so it cannot head-of-line
                # block the input prefetch descriptors on the sync ring
                nc.scalar.dma_start(out_d.ap()[j], out_sb[:])

    nc.compile()
    return nc


def _prep_core(x, batch, bounds, c, T, TB, G_BLK):
    """Per-core padded shards in the three device layouts + per-block bases."""
    s, e = int(bounds[c]), int(bounds[c + 1])
    n = e - s
    B = T // TB
    w = TB * P
    x_c = np.zeros((T * P, HDIM), dtype=np.float32)
    x_c[:n] = x[s:e]
    # node-major [x | 1] bf16: [B, P, TB*(HDIM+1)]
    xn1 = np.ones((T * P, HDIM + 1), dtype=_NP_BF)
    xn1[:, :HDIM] = x_c.astype(_NP_BF)
    xn = np.ascontiguousarray(
        xn1.reshape(B, TB, P, HDIM + 1).transpose(0, 2, 1, 3)
    ).reshape(B, P, TB * (HDIM + 1))
    # feature-major fp8: [B, P, 2*TB*P]
    xtq = np.ascontiguousarray(
        x_c.astype(_NP_F8).reshape(B, TB * P, 2, HHID).transpose(0, 3, 2, 1)
    ).reshape(B, P, 2 * TB * P)
    lidx = np.full(T * P, -1, dtype=np.int64)
    g0 = np.zeros(B, dtype=np.int64)
    bl = batch[s:e]
    for j in range(B):
        lo = j * w
        hi = min(lo + w, n)
        if hi <= lo:
            g0[j] = int(batch[e - 1]) if n > 0 else 0
            continue
        g0[j] = int(bl[lo])
        lidx[lo:hi] = bl[lo:hi] - g0[j]
    lidx_b = np.ascontiguousarray(
        lidx.astype(_NP_BF).reshape(B, TB, P).transpose(0, 2, 1)
    )
    return xn, xtq, lidx_b, g0


def _make_consts(W1, b1, W2, G_BLK):
    cf = np.zeros((P, 1), dtype=np.float32)
    cf[:, 0] = b1
    cb = np.zeros((P, G_BLK + 1), dtype=_NP_BF)
    cb[:, 0:G_BLK] = np.arange(G_BLK, dtype=np.float32)[None, :].astype(_NP_BF)
    cb[:, G_BLK] = W2[:, 0].astype(_NP_BF)
    cq = np.zeros((P, 2 * HHID), dtype=_NP_F8)
    cq[:, 0:HHID] = W1[:HHID, :].astype(_NP_F8)
    cq[:, HHID : 2 * HHID] = W1[HHID:, :].astype(_NP_F8)
    return cf, cb, cq


_CACHE = {}


def _get_program(T, TB, G_BLK, B):
    key = (T, TB, G_BLK, B)
    if key not in _CACHE:
        _CACHE[key] = _build_program(T, TB, G_BLK, B)
    return _CACHE[key]


def build_in_maps(x, W1, b1, W2, batch):
    """Host-side prep shared by kernel() and the timing harness."""
    batch = np.asarray(batch, dtype=np.int64)
    x = np.asarray(x, dtype=np.float32)
    bounds, TB, G_BLK, B, T = _plan(batch)
    cf, cb, cq = _make_consts(
        np.asarray(W1, dtype=np.float32),
        np.asarray(b1, dtype=np.float32),
        np.asarray(W2, dtype=np.float32),
        G_BLK,
    )
    in_maps, g0s = [], []
    for c in range(N_CORES):
        xn, xtq, lidx_b, g0 = _prep_core(x, batch, bounds, c, T, TB, G_BLK)
        in_maps.append(
            {"xn": xn, "xtq": xtq, "lidx": lidx_b, "cf": cf, "cb": cb, "cq": cq}
        )
        g0s.append(g0)
    return in_maps, g0s, (T, TB, G_BLK, B)


def combine(results, g0s, G_BLK):
    """Sum per-block partials into the global output and normalize."""
    U = np.zeros((NUM_GRAPHS + G_BLK, HDIM), dtype=np.float64)
    S = np.zeros(NUM_GRAPHS + G_BLK, dtype=np.float64)
    for out_c, g0 in zip(results, g0s):
        npart = out_c.shape[1] // G_BLK
        oc = out_c.reshape(out_c.shape[0], npart, G_BLK, HDIM + 1).sum(axis=1)
        for j in range(oc.shape[0]):
            g = int(g0[j])
            U[g : g + G_BLK] += oc[j, :, :HDIM]
            S[g : g + G_BLK] += oc[j, :, HDIM]
    return (U[:NUM_GRAPHS] / (S[:NUM_GRAPHS, None] + 1e-16)).astype(np.float32)


def kernel(x, W1, b1, W2, b2, batch):
    in_maps, g0s, (T, TB, G_BLK, B) = build_in_maps(x, W1, b1, W2, batch)
    nc = _get_program(T, TB, G_BLK, B)
    res = run_bass_kernel_spmd(nc, in_maps, core_ids=list(range(N_CORES)))
    outs = [res.results[c]["out"] for c in range(N_CORES)]
    return combine(outs, g0s, G_BLK)
